# revision 28
# baseline (speedup 1.0000x reference)
"""Trainium2 Bass kernel for column-softmax attention.

reference semantics:
    scores = einsum('bqd,bkd->bqk', q, k) / sqrt(128)   # [B, Nq, Nk]
    attn   = softmax(scores, axis=1)                     # over the QUERY axis
    out    = einsum('bqk,bkd->bqd', attn, v)             # [B, Nq, D]

Because the softmax is over q, each key column normalizes independently:
    out[q, d] = sum_k E[k, q] * r[k] * v[k, d],  E = exp(scores.T), r = 1/sum_q E[k, q]

Sharding: 8 cores = 4 batches x 2 key-halves.  Each core computes the partial
sum over its 2048 keys; the host adds the two partials per batch.

v2 design (ACT was the bottleneck at 72.5us busy; the exp element count per
core is 8.4M = 54.6us of pure ACT cycles):

- Per key tile the 4096 score columns split 1536(ACT) + 1536(ACT) +
  2x512(DVE).  The DVE computes exp with the Schraudolph bit trick: the
  fp16 bit pattern of exp(x) is approximately round(x*1024*log2(e) + 15360+c),
  so one tensor_scalar (mult,add) from fp32 PSUM scores to an int16 view of
  the fp16 E tile produces exp in a single 1x DVE pass (rms rel err ~1.8%
  on the offloaded quarter of columns; RNE convert verified on HW).
  Row-sums for the DVE columns come from one 4x-mode tensor_scalar with
  accum_out over the packed fp16 E range.  ACT chunks keep accum_out.
  ACT busy drops to ~53us; PE (~58us) becomes the roofline.
- PSUM: one 'S' tag with bufs=2 (2 x 1536 fp32 = 2 x 3 banks); each key
  tile allocates 4 tiles (A,B,C,D) so the rotation parity is stable and
  ACT alternates buffers with no write-after-read bubble.  AV keeps 2
  banks (tag 'O', bufs=2).
- AV grouped [[0],[1-6],[7-12],[13,14],[15]]: a group's AV matmuls +
  PSUM-accumulate run during the NEXT group's window, cutting DVE
  flush-add passes to 5 (each pass is 8 chunk-adds of 512 at 1x).
  Group 0's flush is a plain copy and runs on ScalarE (it has slack).
- Key tile 0 splits its first ACT chunk into 3x512 so the first exp
  starts as soon as the first 512-column score matmul lands (~2us).
"""

import numpy as np

import concourse.bass as bass
import concourse.mybir as mybir
import concourse.tile as tile
from concourse.bass_utils import run_bass_kernel_spmd

B, N, D = 4, 4096, 128
P = 128
NK = 2048                 # keys per core (half of 4096)
KT_TILES = NK // P        # 16 key tiles of 128
SCALE = 1.0 / np.sqrt(128.0)

F32 = mybir.dt.float32
F16 = mybir.dt.float16
I16 = mybir.dt.int16

# Schraudolph fp16 exp constants (fit for x ~ N(0,1) scores, RNE convert):
#   I16 = round(x * SCH_A + SCH_B);  fp16bits(I16) ~ exp(x)
# rms rel err 1.77%, max 3.98%, mean bias -4e-4 (c = -59.5).
SCH_A = float(1024.0 * np.log2(np.e) * SCALE)   # fold the 1/sqrt(128) in
SCH_B = float(15360.0 - 59.5)

GROUPS_V2 = [[0], [1, 2, 3, 4, 5, 6], [7, 8, 9, 10, 11, 12], [13, 14, 15]]


def emit_body(nc, tc, pools, aps, groups=None, dve_w=1024, pair_av=True,
              copy_eng="vector", last_act=False, sd_pool="x",
              rowsum_eng="vector", mid_flush="gpsimd"):
    big, inp, epool, small, spsum, opsum = pools
    qt_d, kt_d, v_d, out_d = aps
    act_w = (N - dve_w) // 2          # 1536 for dve_w=1024

    qT = inp.tile([P, N], F16, tag="qT")            # [d, q]
    kT = inp.tile([P, NK], F16, tag="kT")           # [d, k]
    vsb = inp.tile([P, KT_TILES, D], F16, tag="v")  # [k_in_tile, k_tile, d]
    oacc = big.tile([P, N], F32, tag="oacc")        # [d, q] SBUF accumulator
    oacc2 = big.tile([P, N], F32, tag="oacc2")      # mid-cut staging (ACT copy)
    obuf = big.tile([P, N], F16, tag="obuf")        # final sums, fp16 for DMA

    # Input DMAs ordered by first use (HWDGE descriptor generation is
    # serialized, ~625ns each; an early bulk DMA would delay the first
    # q/k chunks that gate the ACT pipeline).
    v_r = v_d.rearrange("(t p) d -> p t d", p=P)
    nc.sync.dma_start(qT[:, 0:512], qt_d[:, 0:512])
    nc.sync.dma_start(kT[:, 0:P], kt_d[:, 0:P])
    nc.sync.dma_start(qT[:, 512:1536], qt_d[:, 512:1536])
    nc.sync.dma_start(qT[:, 1536:3072], qt_d[:, 1536:3072])
    nc.sync.dma_start(qT[:, 3072:4096], qt_d[:, 3072:4096])
    nc.sync.dma_start(vsb[:, 0:1, :], v_r[:, 0:1, :])
    nc.sync.dma_start(kT[:, P:NK], kt_d[:, P:NK])
    nc.sync.dma_start(vsb[:, 1:16, :], v_r[:, 1:16, :])

    # Warm-up matmul: first real matmul then carries at most one sync wait.
    Swarm = spsum.tile([P, act_w], F32, tag="S")
    nc.tensor.matmul(
        Swarm[0:1, 0:1], lhsT=kT[:, 0:1], rhs=kT[:, 0:1], start=True, stop=True
    )

    e_tiles = {}
    vsc_tiles = {}

    # Per-chunk staggered AV flush schedule: chunk c's PSUM accumulation is
    # cut (flushed) at the key tiles in CUTS[c].  Two chunks share each cut
    # key tile (= the 2 AV PSUM banks), so AV matmul work flows at a smooth
    # ~8-10 matmuls per key tile from kt1 onward instead of arriving in
    # group-sized bursts that starve ACT, and the tail is only the short
    # final windows (20 matmuls + 8 flushes).
    # 3 cuts per chunk (24 flush ops instead of 32): on HW the DVE is the
    # binding engine (each op pays a pipeline-drain ~dur-266ns the cost
    # model omits), so fewer flush passes beat smoother PE pacing.
    cuts = {0: [4, 10, 15], 1: [4, 10, 15],
            2: [5, 11, 15], 3: [5, 11, 15],
            4: [6, 13, 15], 5: [6, 13, 15],
            6: [7, 14, 15], 7: [7, 14, 15]}
    prev_cut = {c: -1 for c in cuts}

    def emit_chunk_block(cs_, j):
        """AV for chunks cs_ (1 or 2 sharing the same window) over key tiles
        (prev_cut, j], then flush each.  With 2 chunks the matmuls interleave
        per stationary vsc so each weight load serves both chunks (halves
        LDWEIGHTS traffic on the PE weight port)."""
        kl = list(range(prev_cut[cs_[0]] + 1, j + 1))
        first = prev_cut[cs_[0]] < 0
        last = j == KT_TILES - 1
        p0 = prev_cut[cs_[0]]
        ots = []
        for c in cs_:
            assert prev_cut[c] == p0
            prev_cut[c] = j
            ots.append(opsum.tile([P, 512], F32, tag="O", name=f"Ot{c}"))
        for i, ktg in enumerate(kl):
            for c, Ot in zip(cs_, ots):
                nc.tensor.matmul(
                    Ot[:],
                    lhsT=vsc_tiles[ktg][:],
                    rhs=e_tiles[ktg][:, c * 512: (c + 1) * 512],
                    start=(i == 0),
                    stop=(i == len(kl) - 1),
                )
        for c, Ot in zip(cs_, ots):
            lo = c * 512
            if first:
                if copy_eng == "scalar":
                    nc.scalar.copy(out=oacc[:, lo: lo + 512], in_=Ot[:])
                else:
                    nc.vector.tensor_copy(out=oacc[:, lo: lo + 512], in_=Ot[:])
            elif last:
                nc.vector.tensor_add(
                    obuf[:, lo: lo + 512], Ot[:], oacc[:, lo: lo + 512]
                )
            elif mid_flush == "gpsimd":
                # Middle cut: PSUM->SBUF copy on ScalarE (slack there), then
                # merge into oacc on the idle GPSIMD.  Keeps the mid-kernel
                # DVE queue — the binding resource on HW (drain-laden ops) —
                # free of these flushes entirely.
                nc.scalar.copy(out=oacc2[:, lo: lo + 512], in_=Ot[:])
                nc.gpsimd.tensor_add(
                    oacc[:, lo: lo + 512],
                    oacc2[:, lo: lo + 512],
                    oacc[:, lo: lo + 512],
                )
            else:
                nc.vector.tensor_add(
                    oacc[:, lo: lo + 512], Ot[:], oacc[:, lo: lo + 512]
                )

    N_CHUNK = N // 512

    def emit_cut_blocks(kt, limit=8, strict=False):
        """Emit up to `limit` pending AV blocks whose cut key tile has been
        reached (mid-body calls pass strict=True: only cuts at EARLIER key
        tiles, whose vsc already exists)."""
        hi = kt - 1 if strict else kt
        done = 0
        c = 0
        while c < 8 and done < limit:
            if cuts[c] and cuts[c][0] <= hi and cuts[c][0] < KT_TILES - 1:
                j = cuts[c].pop(0)
                grp = [c]
                if (pair_av and done + 1 < limit and c + 1 < 8 and cuts[c + 1]
                        and cuts[c + 1][0] == j
                        and prev_cut[c + 1] == prev_cut[c]):
                    cuts[c + 1].pop(0)
                    grp.append(c + 1)
                    c += 1
                emit_chunk_block(grp, j)
                done += len(grp)
            c += 1

    def score_mms(kt, S, lo_q, w):
        for u in range(w // 512):
            nc.tensor.matmul(
                S[:, u * 512: (u + 1) * 512],
                lhsT=kT[:, kt * P: (kt + 1) * P],
                rhs=qT[:, lo_q + u * 512: lo_q + u * 512 + 512],
                start=True,
                stop=True,
            )

    def emit_A(kt):
        """Allocate + run the A-chunk score matmuls for key tile kt.
        Emitted at the END of key tile kt-1 (lookahead) so these matmuls
        sit AHEAD of kt-1's AV burst in the PE queue and ACT is never
        starved of its next input."""
        chunks = [(0, act_w)]
        if kt == 0:
            # smaller leading chunks so the first exp starts early
            chunks = [(0, 512), (512, 512), (1024, 512)]
        tiles = []
        for lo_q, w in chunks:
            S = spsum.tile([P, w], F32, tag="S", name=f"SA{kt}")
            score_mms(kt, S, lo_q, w)
            tiles.append((S, lo_q, w))
        return tiles

    A_tiles = emit_A(0)
    for kt in range(KT_TILES):
        E = epool.tile([P, N], F16, tag=f"E{kt}")   # [k, q] = exp(scores.T)
        rs = small.tile([P, 6], F32, tag="rs")
        nrs = 0

        # --- B-chunk score matmuls FIRST: their S buffer was freed when
        # ACT finished reading last key tile's B chunk (just now), and
        # emitting them at the top keeps them ahead of any AV backlog. ---
        SB = spsum.tile([P, act_w], F32, tag="S", name=f"SB{kt}")
        score_mms(kt, SB, act_w, act_w)

        # --- ACT chunk A (matmuls already emitted last key tile) ---
        for S, lo_q, w in A_tiles:
            nc.scalar.activation(
                out=E[:, lo_q: lo_q + w],
                in_=S[:, 0:w],
                func=mybir.ActivationFunctionType.Exp,
                scale=float(SCALE),
                accum_out=rs[:, nrs: nrs + 1],
            )
            nrs += 1

        last_kt = (kt == KT_TILES - 1) and last_act
        dve_lo = 2 * act_w

        if sd_pool == "s":
            # One AV block between the B matmuls and the SD matmuls keeps
            # the PE fed while SD waits for ACT to vacate slot 0.
            emit_cut_blocks(kt, limit=1, strict=True)
            # DVE score chunk lives in the scores pool (slot 0, after A) so
            # the AV banks stay a pure PE-write -> DVE-flush pipeline; a
            # zero-op dummy allocation keeps the 2-buffer parity stable.
            SD = spsum.tile([P, dve_w], F32, tag="S", name=f"SD{kt}")
            dum = spsum.tile([P, 1], F32, tag="S", name=f"Sdum{kt}")
            del dum
            score_mms(kt, SD, dve_lo, dve_w)
            if last_kt:
                nc.scalar.activation(
                    out=E[:, dve_lo: dve_lo + dve_w],
                    in_=SD[:],
                    func=mybir.ActivationFunctionType.Exp,
                    scale=float(SCALE),
                    accum_out=rs[:, nrs: nrs + 1],
                )
                nrs += 1
            else:
                nc.vector.tensor_scalar(
                    out=E[:, dve_lo: dve_lo + dve_w].bitcast(I16),
                    in0=SD[:],
                    scalar1=SCH_A,
                    scalar2=SCH_B,
                    op0=mybir.AluOpType.mult,
                    op1=mybir.AluOpType.add,
                )
        else:
            for half in range(dve_w // 512):
                lo_q = dve_lo + half * 512
                S = opsum.tile([P, 512], F32, tag="O", name=f"SD{half}")
                score_mms(kt, S, lo_q, 512)
                if last_kt:
                    nc.scalar.activation(
                        out=E[:, lo_q: lo_q + 512],
                        in_=S[:],
                        func=mybir.ActivationFunctionType.Exp,
                        scale=float(SCALE),
                        accum_out=rs[:, nrs: nrs + 1],
                    )
                    nrs += 1
                else:
                    nc.vector.tensor_scalar(
                        out=E[:, lo_q: lo_q + 512].bitcast(I16),
                        in0=S[:],
                        scalar1=SCH_A,
                        scalar2=SCH_B,
                        op0=mybir.AluOpType.mult,
                        op1=mybir.AluOpType.add,
                    )

        # --- ACT chunk B ---
        nc.scalar.activation(
            out=E[:, act_w: 2 * act_w],
            in_=SB[:],
            func=mybir.ActivationFunctionType.Exp,
            scale=float(SCALE),
            accum_out=rs[:, nrs: nrs + 1],
        )
        nrs += 1

        if dve_w and not last_kt:
            if rowsum_eng == "gpsimd":
                # Row-sum of the DVE-produced exp columns on the (idle)
                # GPSIMD engine — takes ~5.8us of drain-laden work off DVE.
                nc.gpsimd.reduce_sum(
                    out=rs[:, nrs: nrs + 1],
                    in_=E[:, dve_lo: dve_lo + dve_w],
                    axis=mybir.AxisListType.X,
                )
            else:
                scr = small.tile([P, dve_w], F16, tag="scr", bufs=1)
                nc.vector.tensor_scalar(
                    out=scr[:],
                    in0=E[:, dve_lo: dve_lo + dve_w],
                    scalar1=1.0,
                    scalar2=0.0,
                    op0=mybir.AluOpType.mult,
                    op1=mybir.AluOpType.add,
                    accum_out=rs[:, nrs: nrs + 1],
                )
            nrs += 1

        rsum = small.tile([P, 1], F32, tag="rsum")
        recip = small.tile([P, 1], F32, tag="recip")
        vsc = small.tile([P, D], F16, tag=f"vsc{kt}", bufs=1)
        nc.vector.reduce_sum(
            out=rsum[:], in_=rs[:, 0:nrs], axis=mybir.AxisListType.X
        )
        nc.vector.reciprocal(recip[:], rsum[:])
        nc.vector.tensor_scalar_mul(vsc[:], vsb[:, kt, :], recip[:])
        e_tiles[kt] = E
        vsc_tiles[kt] = vsc

        # Next key tile's A scores BEFORE this tile's AV blocks (see emit_A).
        if kt + 1 < KT_TILES:
            A_tiles = emit_A(kt + 1)
        emit_cut_blocks(kt)

    # Tail: final windows of every chunk (short by construction), unpaired
    # so block i+1's matmuls overlap block i's flush.
    for c in range(8):
        if cuts[c]:
            assert cuts[c] == [KT_TILES - 1]
            emit_chunk_block([c], KT_TILES - 1)
        lo = c * 512
        if c % 2 == 1:
            dlo = (c - 1) * 512
            nc.sync.dma_start(
                out_d[:, dlo: dlo + 1024], obuf[:, dlo: dlo + 1024]
            )


def build_bass(repeat=1, loop=False, **kw):
    nc = bass.Bass("TRN2", target_bir_lowering=False, debug=False)
    qt_d = nc.dram_tensor("qt", [P, N], F16, kind="ExternalInput").ap()
    kt_d = nc.dram_tensor("kt", [P, NK], F16, kind="ExternalInput").ap()
    v_d = nc.dram_tensor("v", [NK, D], F16, kind="ExternalInput").ap()
    out_d = nc.dram_tensor("out_t", [P, N], F16, kind="ExternalOutput").ap()

    with tile.TileContext(nc) as tc:
        with (
            tc.tile_pool(name="big", bufs=1) as big,
            tc.tile_pool(name="inp", bufs=2) as inp,
            tc.tile_pool(name="epool", bufs=1) as epool,
            tc.tile_pool(name="small", bufs=2) as small,
            tc.tile_pool(name="spsum", bufs=2, space="PSUM") as spsum,
            tc.tile_pool(name="opsum", bufs=2, space="PSUM") as opsum,
        ):
            def body():
                emit_body(nc, tc, (big, inp, epool, small, spsum, opsum),
                          (qt_d, kt_d, v_d, out_d), **kw)

            # ACT table preload at t=0, outside the loop: overlaps the first
            # iteration's input DMAs and costs later iterations nothing.
            wrm = big.tile([P, 1], F16, tag="wrm")
            wrmo = big.tile([P, 1], F16, tag="wrmo")
            nc.vector.memset(wrm[:], 0.0)
            nc.scalar.activation(out=wrmo[:], in_=wrm[:],
                                 func=mybir.ActivationFunctionType.Exp, scale=1.0)

            if loop and repeat > 1:
                with tc.For_i(
                    0, repeat, 1,
                    hint_engines=(mybir.EngineType.PE, mybir.EngineType.Activation),
                ):
                    body()
            else:
                for _ in range(repeat):
                    body()
    return nc


def legalize_waits(nc, max_waits=1):
    """Hoist excess semaphore waits into standalone EventSemaphore ops.

    The walrus codegen for several engine instruction structs accepts only a
    single sync-wait command; Tile sometimes emits more.  Executing the extra
    waits in a preceding same-engine EventSemaphore is semantically identical
    (the engine runs its stream in order).
    """
    for fn in nc.m.functions:
        for blk in fn.blocks:
            out = []
            for inst in blk.instructions:
                si = inst.sync_info
                if (
                    si is not None
                    and si.on_wait
                    and len(si.on_wait) > max_waits
                    and inst.opcode != "EventSemaphore"
                ):
                    waits = list(si.on_wait)
                    extra, keep = waits[:-max_waits], waits[-max_waits:]
                    for n, w in enumerate(extra):
                        out.append(
                            mybir.InstEventSemaphore(
                                name=f"{inst.name}_prewait{n}",
                                engine=inst.engine,
                                ins=[],
                                outs=[],
                                sync_info=mybir.SyncInfo(on_wait=[w], on_update=[]),
                            )
                        )
                    si.on_wait = keep
                out.append(inst)
            blk.instructions = out
    return nc


_NC_CACHE = {}


def _get_nc(repeat=1, **kw):
    key = ("nc", repeat, tuple(sorted(kw.items())))
    if key not in _NC_CACHE:
        _NC_CACHE[key] = legalize_waits(build_bass(repeat, **kw))
    return _NC_CACHE[key]


def kernel(q, k, v):
    q = np.asarray(q, dtype=np.float32)
    k = np.asarray(k, dtype=np.float32)
    v = np.asarray(v, dtype=np.float32)

    in_maps = []
    for c in range(8):
        b, h = c // 2, c % 2
        in_maps.append(
            {
                "qt": np.ascontiguousarray(q[b].T).astype(np.float16),
                "kt": np.ascontiguousarray(k[b, h * NK: (h + 1) * NK].T).astype(np.float16),
                "v": np.ascontiguousarray(v[b, h * NK: (h + 1) * NK]).astype(np.float16),
            }
        )

    nc = _get_nc()
    res = run_bass_kernel_spmd(nc, in_maps, list(range(8))).results

    out = np.empty((B, N, D), dtype=np.float32)
    for b in range(B):
        out[b] = (
            res[2 * b]["out_t"].astype(np.float32)
            + res[2 * b + 1]["out_t"].astype(np.float32)
        ).T
    return out
